# revision 1
# baseline (speedup 1.0000x reference)
"""ChannelAttention TRN2 kernel.

Math (per token t, head h; hd=16):
  qkv = x @ w_qkv + b_qkv ; q,k,v = split(qkv)
  A[i,j] = softmax_j( scale * q[t,h,i] * k[t,h,j] )
  out[t,h,i] = sum_j A[i,j] v[t,h,j] ;  y = out @ w_proj + b_proj

Everything is per-token, so the 65536 tokens are sharded 8 ways with
weights replicated; no collectives.

Device layout: tokens on SBUF partitions (128/tile), channels/softmax
(h,i,j) along the free dim.  x is passed in pre-transposed [C, T] so the
qkv matmul can use the x tile as the stationary operand and produce
token-major qkv directly; output is produced channel-major [C, T] and
transposed back on the host.

Perf structure per 256-token flight:
  - PE: qkv matmul (bf16), attn transpose, proj matmul
  - DVE: bias adds, z=q*k outer product (bf16 2x), E*v (bf16 2x),
    pairwise-add reduction trees over j (bf16 2x), recip + scale
  - ACT: the exp (the only engine that can do it)
  - GpSimd: replicates q along j so the z outer product runs at 2x
"""

import numpy as np

B, L, C = 4, 16384, 128
H, HD = 8, 16
NCORES = 8
NTOK = B * L
TPC = NTOK // NCORES  # 8192 tokens per core
SCALE = float(C) ** -0.5

FL = 256          # tokens per flight
SUB = FL // 128   # 128-token subtiles per flight
NFL = TPC // FL

_BUILT = None
_LAST_IN_MAPS = None


def _build(repeat=1):
    import concourse.bass as bass
    from concourse import bacc
    from concourse import mybir
    from concourse.tile import TileContext
    from concourse.masks import make_identity
    from contextlib import ExitStack

    f32 = mybir.dt.float32
    bf16 = mybir.dt.bfloat16

    nc = bacc.Bacc("TRN2")
    xT = nc.dram_tensor("xT", [C, TPC], bf16, kind="ExternalInput")
    w_qkv = nc.dram_tensor("w_qkv", [C, 3 * C], f32, kind="ExternalInput")
    b_qkv = nc.dram_tensor("b_qkv", [3 * C], f32, kind="ExternalInput")
    w_proj = nc.dram_tensor("w_proj", [C, C], f32, kind="ExternalInput")
    b_proj = nc.dram_tensor("b_proj", [C], f32, kind="ExternalInput")
    outT = nc.dram_tensor("outT", [C, TPC], f32, kind="ExternalOutput")

    with TileContext(nc) as tc, ExitStack() as ctx:
        consts = ctx.enter_context(tc.tile_pool(name="consts", bufs=1))
        qkvp = ctx.enter_context(tc.tile_pool(name="qkvp", bufs=2, space="PSUM"))
        tpp = ctx.enter_context(tc.tile_pool(name="tpp", bufs=2, space="PSUM"))
        qkvs = ctx.enter_context(tc.tile_pool(name="qkvs", bufs=3))
        zpool = ctx.enter_context(tc.tile_pool(name="zpool", bufs=2))
        epool = ctx.enter_context(tc.tile_pool(name="epool", bufs=2))
        evpool = ctx.enter_context(tc.tile_pool(name="evpool", bufs=2))
        trpool = ctx.enter_context(tc.tile_pool(name="trpool", bufs=2))
        ndpool = ctx.enter_context(tc.tile_pool(name="ndpool", bufs=3))
        atpool = ctx.enter_context(tc.tile_pool(name="atpool", bufs=3))
        ypool = ctx.enter_context(tc.tile_pool(name="ypool", bufs=2, space="PSUM"))

        # ---- constants ----
        wqkv_f = consts.tile([C, 3 * C], f32)
        nc.sync.dma_start(out=wqkv_f, in_=w_qkv[:, :])
        wqkv_bf = consts.tile([C, 3 * C], bf16)
        nc.vector.tensor_copy(wqkv_bf[:], wqkv_f[:])

        wp_f = consts.tile([C, C], f32)
        nc.sync.dma_start(out=wp_f, in_=w_proj[:, :])
        wp_bf = consts.tile([C, C], bf16)
        nc.vector.tensor_copy(wp_bf[:], wp_f[:])

        # qkv bias replicated across all 128 partitions (token-major layout)
        bqkv_rep = consts.tile([128, 3 * C], f32)
        nc.sync.dma_start(
            out=bqkv_rep, in_=b_qkv[:].unsqueeze(0).broadcast_to((128, 3 * C))
        )
        # proj bias: per-partition [C,1] (channel-major layout)
        bp_sb = consts.tile([C, 1], f32)
        nc.sync.dma_start(out=bp_sb, in_=b_proj[:].unsqueeze(1))

        ident = consts.tile([128, 128], bf16)
        make_identity(nc, ident[:])

        from contextlib import nullcontext

        rep_ctx = tc.For_i(0, repeat, 1) if repeat > 1 else nullcontext()
        with rep_ctx:
            _emit_flights(
                nc, tc, mybir, consts, qkvp, tpp, qkvs, zpool, epool,
                evpool, trpool, ndpool, atpool, ypool,
                wqkv_bf, wp_bf, bqkv_rep, bp_sb, ident, xT, outT,
            )

    nc.compile()
    return nc


def _emit_flights(
    nc, tc, mybir, consts, qkvp, tpp, qkvs, zpool, epool,
    evpool, trpool, ndpool, atpool, ypool,
    wqkv_bf, wp_bf, bqkv_rep, bp_sb, ident, xT, outT,
):
    f32 = mybir.dt.float32
    bf16 = mybir.dt.bfloat16
    if True:
        for fi in range(NFL):
            t0 = fi * FL
            xt = qkvs.tile([C, FL], bf16, tag="xt")
            nc.sync.dma_start(out=xt, in_=xT[:, t0 : t0 + FL])

            # qkv matmul: x-subtile stationary -> token-major qkv (psum f32)
            # bias-add splits into separate q/k/v staging tiles so all
            # softmax APs stay within the ISA's 3-free-dim limit
            q_sm = qkvs.tile([128, SUB, C], bf16, tag="qsm")
            k_sm = qkvs.tile([128, SUB, C], bf16, tag="ksm")
            v_sm = qkvs.tile([128, SUB, C], bf16, tag="vsm")
            for s in range(SUB):
                ps = qkvp.tile([128, 3 * C], f32, tag="qkvps")
                nc.tensor.matmul(
                    out=ps[:],
                    lhsT=xt[:, s * 128 : (s + 1) * 128],
                    rhs=wqkv_bf[:],
                    start=True,
                    stop=True,
                )
                for g, sm in enumerate((q_sm, k_sm, v_sm)):
                    nc.vector.tensor_add(
                        sm[:, s, :],
                        ps[:, g * C : (g + 1) * C],
                        bqkv_rep[:, g * C : (g + 1) * C],
                    )

            # [128, SUB*H, HD] views (contiguous, so (s h) merges into one dim)
            qm = q_sm.rearrange("p s (h d) -> p (s h) d", h=H)
            km = k_sm.rearrange("p s (h d) -> p (s h) d", h=H)
            vm = v_sm.rearrange("p s (h d) -> p (s h) d", h=H)
            SH = SUB * H
            k_ap = km.unsqueeze(2).broadcast_to((128, SH, HD, HD))
            v_ap = vm.unsqueeze(2).broadcast_to((128, SH, HD, HD))

            # z[i,j] = q_i * k_j  (scale folded into exp)
            z = zpool.tile([128, SH, HD, HD], bf16, tag="z")
            nc.vector.tensor_mul(
                z[:], qm.unsqueeze(3).broadcast_to((128, SH, HD, HD)), k_ap
            )

            e = epool.tile([128, SH, HD, HD], bf16)
            nc.scalar.activation(
                e[:], z[:], mybir.ActivationFunctionType.Exp, scale=SCALE
            )

            ev = evpool.tile([128, SH, HD, HD], bf16, tag="ev")
            nc.vector.tensor_mul(ev[:], e[:], v_ap)

            # pairwise-add trees over j: 16 -> 8 -> 4 -> 2 -> 1 (final level f32)
            def tree(src, tag):
                w = HD
                cur = src
                while w > 2:
                    nxt = trpool.tile([128, SH, HD, w // 2], bf16, tag=f"{tag}{w}")
                    nc.vector.tensor_add(
                        nxt[:],
                        cur[:, :, :, 0 : w // 2],
                        cur[:, :, :, w // 2 : w],
                    )
                    cur = nxt
                    w //= 2
                res = ndpool.tile([128, SH, HD], f32, tag=f"{tag}f")
                nc.vector.tensor_add(res[:], cur[:, :, :, 0], cur[:, :, :, 1])
                return res

            d = tree(e, "d")
            n = tree(ev, "n")

            rd = ndpool.tile([128, SH, HD], f32, tag="rd")
            nc.vector.reciprocal_approx_fast(out=rd[:], in_=d[:])

            at_a = atpool.tile([128, SUB, C], bf16, tag="ata")
            nc.vector.tensor_mul(
                at_a.rearrange("p s (h d) -> p (s h) d", h=H), n[:], rd[:]
            )

            # transpose to channel-major for the proj matmul (PE transpose)
            at_b = atpool.tile([C, FL], bf16, tag="atb")
            for s in range(SUB):
                tp = tpp.tile([128, 128], bf16, tag="tps")
                nc.tensor.transpose(tp[:], at_a[:, s, :], ident[:])
                nc.vector.tensor_copy(at_b[:, s * 128 : (s + 1) * 128], tp[:])

            yp = ypool.tile([C, FL], f32)
            nc.tensor.matmul(
                out=yp[:], lhsT=wp_bf[:], rhs=at_b[:], start=True, stop=True
            )
            y = atpool.tile([C, FL], f32, tag="y")
            nc.vector.tensor_scalar_add(y[:], yp[:], bp_sb[:])
            nc.sync.dma_start(out=outT[:, t0 : t0 + FL], in_=y[:])


def kernel(x, w_qkv, b_qkv, w_proj, b_proj):
    from concourse import bass_utils

    global _BUILT
    if _BUILT is None:
        _BUILT = _build()
    nc = _BUILT

    import ml_dtypes

    xf = np.asarray(x, np.float32).reshape(NTOK, C)
    w_qkv = np.ascontiguousarray(np.asarray(w_qkv, np.float32))
    b_qkv = np.ascontiguousarray(np.asarray(b_qkv, np.float32))
    w_proj = np.ascontiguousarray(np.asarray(w_proj, np.float32))
    b_proj = np.ascontiguousarray(np.asarray(b_proj, np.float32))

    in_maps = []
    for i in range(NCORES):
        shard = xf[i * TPC : (i + 1) * TPC]
        in_maps.append(
            {
                "xT": np.ascontiguousarray(shard.T).astype(ml_dtypes.bfloat16),
                "w_qkv": w_qkv,
                "b_qkv": b_qkv,
                "w_proj": w_proj,
                "b_proj": b_proj,
            }
        )

    global _LAST_IN_MAPS
    _LAST_IN_MAPS = in_maps
    res = bass_utils.run_bass_kernel_spmd(nc, in_maps, core_ids=list(range(NCORES)))
    y = np.concatenate(
        [np.asarray(res.results[i]["outT"]).T for i in range(NCORES)], axis=0
    )
    return y.reshape(B, L, C)



# revision 3
# speedup vs baseline: 29.2860x; 29.2860x over previous
"""ChannelAttention TRN2 kernel.

Math (per token t, head h; hd=16):
  qkv = x @ w_qkv + b_qkv ; q,k,v = split(qkv)
  A[i,j] = softmax_j( scale * q[t,h,i] * k[t,h,j] )
  out[t,h,i] = sum_j A[i,j] v[t,h,j] ;  y = out @ w_proj + b_proj

65536 tokens sharded 8 ways, weights replicated, no collectives.
Tokens on SBUF partitions (128/tile), (h,i,j) along the free dim.

Engine split per 256-token flight (cost-model ns):
  PE   : rank-1 bias-fold matmuls + qkv matmul (per subtile), 2
         transposes, rank-1 proj-bias fold + proj matmul
  ACT  : q broadcast-replication along j for subtile 0 (1849), k/v/q
         staging copies PSUM->SBUF bf16, exp (3598), transpose copies,
         y copy
  DVE  : z=q*k 2x (2193), ev=e*v 2x (2193), merged e/ev pairwise tree
         L1 (2193) + L2 (1127), recip (327)
  Pool : q replication for subtile 1 from SBUF (2939), tree L3 (2126) +
         L4 f32 (1110), at = N*(1/D) (603)
All bf16 operands packed on the last dim so DVE TT ops hit 2x mode; the
replicated q operand of the outer product is materialized by ACT/Pool so
no stride-0 AP reaches DVE.  Biases ride into PSUM via rank-1 matmuls
(ones ⊗ bias) so no DVE bias adds exist.  Flights are software-pipelined
3 deep (headA | headB | tail) to kill cross-engine head-of-line stalls.
"""

import numpy as np

B, L, C = 4, 16384, 128
H, HD = 8, 16
NCORES = 8
NTOK = B * L
TPC = NTOK // NCORES  # 8192 tokens per core
SCALE = float(C) ** -0.5

FL = 256          # tokens per flight
SUB = FL // 128   # 128-token subtiles per flight
SH = SUB * H      # 16
NFL = TPC // FL   # 32

_BUILT = None
_LAST_IN_MAPS = None


def _build(repeat=1):
    import concourse.bass as bass
    from concourse import bacc
    from concourse import mybir
    from concourse.tile import TileContext
    from concourse.masks import make_identity
    from contextlib import ExitStack, nullcontext

    f32 = mybir.dt.float32
    bf16 = mybir.dt.bfloat16

    nc = bacc.Bacc("TRN2")
    xT = nc.dram_tensor("xT", [C, TPC], bf16, kind="ExternalInput")
    w_qkv = nc.dram_tensor("w_qkv", [C, 3 * C], f32, kind="ExternalInput")
    b_qkv = nc.dram_tensor("b_qkv", [3 * C], f32, kind="ExternalInput")
    w_proj = nc.dram_tensor("w_proj", [C, C], f32, kind="ExternalInput")
    b_proj = nc.dram_tensor("b_proj", [C], f32, kind="ExternalInput")
    outT = nc.dram_tensor("outT", [C, TPC], f32, kind="ExternalOutput")

    with TileContext(nc) as tc, ExitStack() as ctx:
        consts = ctx.enter_context(tc.tile_pool(name="consts", bufs=1))
        qkvp = ctx.enter_context(tc.tile_pool(name="qkvp", bufs=3, space="PSUM"))
        tpp = ctx.enter_context(tc.tile_pool(name="tpp", bufs=2, space="PSUM"))
        ypool = ctx.enter_context(tc.tile_pool(name="ypool", bufs=2, space="PSUM"))
        stg = ctx.enter_context(tc.tile_pool(name="stg", bufs=4))
        qrp = ctx.enter_context(tc.tile_pool(name="qrp", bufs=3))
        zp = ctx.enter_context(tc.tile_pool(name="zp", bufs=3))
        mp = ctx.enter_context(tc.tile_pool(name="mp", bufs=3))
        trp = ctx.enter_context(tc.tile_pool(name="trp", bufs=DELAY + 2))
        fp = ctx.enter_context(tc.tile_pool(name="fp", bufs=DELAY + 2))
        apo = ctx.enter_context(tc.tile_pool(name="apo", bufs=3))

        # ---- constants ----
        wqkv_f = consts.tile([C, 3 * C], f32)
        nc.sync.dma_start(out=wqkv_f, in_=w_qkv[:, :])
        wqkv_bf = consts.tile([C, 3 * C], bf16)
        nc.vector.tensor_copy(wqkv_bf[:], wqkv_f[:])

        wp_f = consts.tile([C, C], f32)
        nc.sync.dma_start(out=wp_f, in_=w_proj[:, :])
        wp_bf = consts.tile([C, C], bf16)
        nc.vector.tensor_copy(wp_bf[:], wp_f[:])

        # row vectors (single partition) for the rank-1 bias-fold matmuls
        bqkv_row_f = consts.tile([1, 3 * C], f32)
        nc.sync.dma_start(out=bqkv_row_f, in_=b_qkv[:].unsqueeze(0))
        bqkv_row = consts.tile([1, 3 * C], bf16)
        nc.vector.tensor_copy(bqkv_row[:], bqkv_row_f[:])

        bp_row_f = consts.tile([1, C], f32)
        nc.sync.dma_start(out=bp_row_f, in_=b_proj[:].unsqueeze(0))
        bp_row = consts.tile([1, C], bf16)
        nc.vector.tensor_copy(bp_row[:], bp_row_f[:])

        ones_row = consts.tile([1, FL], bf16)
        nc.vector.memset(ones_row[:], 1.0)

        ident = consts.tile([128, 128], bf16)
        make_identity(nc, ident[:])

        rep_ctx = tc.For_i(0, repeat, 1) if repeat > 1 else nullcontext()
        with rep_ctx:
            _emit_flights(
                nc, mybir, qkvp, tpp, ypool, stg, qrp, zp, mp, trp, fp, apo,
                wqkv_bf, wp_bf, bqkv_row, bp_row, ones_row, ident, xT, outT,
            )

    nc.compile()
    return nc


DELAY = 2  # flights between a flight's head (softmax) and tail (proj+out)


def _emit_flights(
    nc, mybir, qkvp, tpp, ypool, stg, qrp, zp, mp, trp, fp, apo,
    wqkv_bf, wp_bf, bqkv_row, bp_row, ones_row, ident, xT, outT,
):
    f32 = mybir.dt.float32
    bf16 = mybir.dt.bfloat16
    Exp = mybir.ActivationFunctionType.Exp
    Copy = mybir.ActivationFunctionType.Copy

    def head(fi):
        t0 = fi * FL
        xt = stg.tile([C, FL], bf16, tag="xt")
        nc.sync.dma_start(out=xt, in_=xT[:, t0 : t0 + FL])

        k_sm = stg.tile([128, SUB, H, HD], bf16, tag="ksm")
        v_sm = stg.tile([128, SUB, H, HD], bf16, tag="vsm")
        q_sm = stg.tile([128, H, HD], bf16, tag="qsm")
        qrep = qrp.tile([128, SH, HD, HD], bf16, tag="qrep")

        for s in range(SUB):
            # qkv matmul with bias folded in via a rank-1 PSUM init
            ps = qkvp.tile([128, 3, H, HD], f32, tag="ps")
            nc.tensor.matmul(
                out=ps[:],
                lhsT=ones_row[:, 0:128],
                rhs=bqkv_row[:],
                start=True,
                stop=False,
            )
            nc.tensor.matmul(
                out=ps[:],
                lhsT=xt[:, s * 128 : (s + 1) * 128],
                rhs=wqkv_bf[:],
                start=False,
                stop=True,
            )
            # replicate q along j: subtile 0 by ACT straight from PSUM;
            # subtile 1 by GPSIMD from a small SBUF staging copy (no PSUM
            # port on GPSIMD)
            if s == 0:
                nc.scalar.activation(
                    qrep[:, s * H : (s + 1) * H],
                    ps[:, 0].unsqueeze(3).broadcast_to((128, H, HD, HD)),
                    Copy,
                )
            else:
                nc.scalar.copy(q_sm[:], ps[:, 0])
                nc.gpsimd.tensor_copy(
                    qrep[:, s * H : (s + 1) * H],
                    q_sm.unsqueeze(3).broadcast_to((128, H, HD, HD)),
                )
            # stage k, v (ACT) to SBUF bf16
            nc.scalar.copy(k_sm[:, s], ps[:, 1])
            nc.scalar.copy(v_sm[:, s], ps[:, 2])

        km = k_sm.rearrange("p s h j -> p (s h) j")
        k_ap = km.unsqueeze(2).broadcast_to((128, SH, HD, HD))

        # z[i,j] = q_i * k_j  (scale folded into exp); qrep packed -> 2x
        z = zp.tile([128, SH, HD, HD], bf16, tag="z")
        nc.vector.tensor_mul(z[:], qrep[:], k_ap)
        return z, v_sm

    def headB(state):
        z, v_sm = state
        vm = v_sm.rearrange("p s h j -> p (s h) j")
        v_ap = vm.unsqueeze(2).broadcast_to((128, SH, HD, HD))

        # M[:,0] = e = exp(scale*z) ; M[:,1] = ev = e * v
        M = mp.tile([128, 2, SH, HD, HD], bf16, tag="m")
        nc.scalar.activation(M[:, 0], z[:], Exp, scale=SCALE)
        nc.vector.tensor_mul(M[:, 1], M[:, 0], v_ap)

        # merged pairwise-add tree over j for both e (->D) and ev (->N)
        Mv = M.rearrange("p c s i j -> p (c s) i j")
        T1 = trp.tile([128, 2 * SH, HD, 8], bf16, tag="t1")
        nc.vector.tensor_add(T1[:], Mv[:, :, :, 0:8], Mv[:, :, :, 8:16])
        T2 = trp.tile([128, 2 * SH, HD, 4], bf16, tag="t2")
        nc.vector.tensor_add(T2[:], T1[:, :, :, 0:4], T1[:, :, :, 4:8])
        return T2

    def tail(fi, T2):
        t0 = fi * FL
        T3 = trp.tile([128, 2 * SH, HD, 2], bf16, tag="t3")
        nc.gpsimd.tensor_add(T3[:], T2[:, :, :, 0:2], T2[:, :, :, 2:4])
        F = fp.tile([128, 2 * SH, HD], f32, tag="f")
        nc.gpsimd.tensor_add(F[:], T3[:, :, :, 0], T3[:, :, :, 1])
        # at = N / D  (recip on DVE, multiply on GPSIMD)
        rd = fp.tile([128, SH, HD], f32, tag="rd")
        nc.vector.reciprocal_approx_fast(out=rd[:], in_=F[:, 0:SH])
        at_a = apo.tile([128, SUB, C], bf16, tag="ata")
        nc.gpsimd.tensor_mul(
            at_a.rearrange("p s (h i) -> p (s h) i", h=H), F[:, SH : 2 * SH], rd[:]
        )

        # transpose to channel-major, proj matmul with bias folded, DMA out
        at_b = apo.tile([C, FL], bf16, tag="atb")
        for s in range(SUB):
            tp = tpp.tile([128, 128], bf16, tag="tp")
            nc.tensor.transpose(tp[:], at_a[:, s, :], ident[:])
            nc.scalar.copy(at_b[:, s * 128 : (s + 1) * 128], tp[:])

        yp = ypool.tile([C, FL], f32, tag="yp")
        nc.tensor.matmul(
            out=yp[:], lhsT=bp_row[:], rhs=ones_row[:], start=True, stop=False
        )
        nc.tensor.matmul(
            out=yp[:], lhsT=wp_bf[:], rhs=at_b[:], start=False, stop=True
        )
        y = apo.tile([C, FL], f32, tag="y")
        nc.scalar.copy(y[:], yp[:])
        nc.sync.dma_start(out=outT[:, t0 : t0 + FL], in_=y[:])

    # 3-stage software pipeline: headA(f) | headB(f-1) | tail(f-2)
    pendB = []
    pendT = []
    for fi in range(NFL):
        pendB.append((fi, head(fi)))
        if len(pendB) > 1:
            g, st = pendB.pop(0)
            pendT.append((g, headB(st)))
        if len(pendT) >= DELAY:
            tail(*pendT.pop(0))
    while pendB:
        g, st = pendB.pop(0)
        pendT.append((g, headB(st)))
    while pendT:
        tail(*pendT.pop(0))


def kernel(x, w_qkv, b_qkv, w_proj, b_proj):
    from concourse import bass_utils

    global _BUILT
    if _BUILT is None:
        _BUILT = _build()
    nc = _BUILT

    import ml_dtypes

    xf = np.asarray(x, np.float32).reshape(NTOK, C)
    w_qkv = np.ascontiguousarray(np.asarray(w_qkv, np.float32))
    b_qkv = np.ascontiguousarray(np.asarray(b_qkv, np.float32))
    w_proj = np.ascontiguousarray(np.asarray(w_proj, np.float32))
    b_proj = np.ascontiguousarray(np.asarray(b_proj, np.float32))

    in_maps = []
    for i in range(NCORES):
        shard = xf[i * TPC : (i + 1) * TPC]
        in_maps.append(
            {
                "xT": np.ascontiguousarray(shard.T).astype(ml_dtypes.bfloat16),
                "w_qkv": w_qkv,
                "b_qkv": b_qkv,
                "w_proj": w_proj,
                "b_proj": b_proj,
            }
        )

    global _LAST_IN_MAPS
    _LAST_IN_MAPS = in_maps
    res = bass_utils.run_bass_kernel_spmd(nc, in_maps, core_ids=list(range(NCORES)))
    y = np.concatenate(
        [np.asarray(res.results[i]["outT"]).T for i in range(NCORES)], axis=0
    )
    return y.reshape(B, L, C)


# revision 7
# speedup vs baseline: 40.8841x; 1.3960x over previous
"""ChannelAttention TRN2 kernel.

Math (per token t, head h; hd=16):
  qkv = x @ w_qkv + b_qkv ; q,k,v = split(qkv)
  A[i,j] = softmax_j( scale * q[t,h,i] * k[t,h,j] )
  out[t,h,i] = sum_j A[i,j] v[t,h,j] ;  y = out @ w_proj + b_proj

65536 tokens sharded 8 ways, weights replicated, no collectives.
Tokens on SBUF partitions (128/tile), (h,i,j) along the free dim.

Engine split per 256-token flight (cost-model ns):
  PE   : rank-1 bias-fold matmuls + qkv matmul (per subtile), 2
         transposes, rank-1 proj-bias fold + proj matmul
  ACT  : q broadcast-replication along j for subtile 0 (1849), k/v/q
         staging copies PSUM->SBUF bf16, exp (3598), transpose copies,
         y copy
  DVE  : z=q*k 2x (2193), ev=e*v 2x (2193), merged e/ev pairwise tree
         L1 (2193) + L2 (1127), recip (327)
  Pool : q replication for subtile 1 from SBUF (2939), tree L3 (2126) +
         L4 f32 (1110), at = N*(1/D) (603)
All bf16 operands packed on the last dim so DVE TT ops hit 2x mode; the
replicated q operand of the outer product is materialized by ACT/Pool so
no stride-0 AP reaches DVE.  Biases ride into PSUM via rank-1 matmuls
(ones ⊗ bias) so no DVE bias adds exist.  Flights are software-pipelined
3 deep (headA | headB | tail) to kill cross-engine head-of-line stalls.
"""

import numpy as np

B, L, C = 4, 16384, 128
H, HD = 8, 16
NCORES = 8
NTOK = B * L
TPC = NTOK // NCORES  # 8192 tokens per core
SCALE = float(C) ** -0.5

FL = 256          # tokens per flight
SUB = FL // 128   # 128-token subtiles per flight
SH = SUB * H      # 16
NFL = TPC // FL   # 32

_BUILT = None
_LAST_IN_MAPS = None


def _build(repeat=1):
    import concourse.bass as bass
    from concourse import bacc
    from concourse import mybir
    from concourse.tile import TileContext
    from concourse.masks import make_identity
    from contextlib import ExitStack, nullcontext

    f32 = mybir.dt.float32
    bf16 = mybir.dt.bfloat16

    nc = bacc.Bacc("TRN2")
    xT = nc.dram_tensor("xT", [C, TPC], bf16, kind="ExternalInput")
    w_qkv = nc.dram_tensor("w_qkv", [C, 3 * C], f32, kind="ExternalInput")
    b_qkv = nc.dram_tensor("b_qkv", [3 * C], f32, kind="ExternalInput")
    w_proj = nc.dram_tensor("w_proj", [C, C], f32, kind="ExternalInput")
    b_proj = nc.dram_tensor("b_proj", [C], f32, kind="ExternalInput")
    outT = nc.dram_tensor("outT", [C, TPC], f32, kind="ExternalOutput")

    with TileContext(nc) as tc, ExitStack() as ctx:
        consts = ctx.enter_context(tc.tile_pool(name="consts", bufs=1))
        qkvp = ctx.enter_context(tc.tile_pool(name="qkvp", bufs=3, space="PSUM"))
        tpp = ctx.enter_context(tc.tile_pool(name="tpp", bufs=2, space="PSUM"))
        ypool = ctx.enter_context(tc.tile_pool(name="ypool", bufs=2, space="PSUM"))
        stg = ctx.enter_context(tc.tile_pool(name="stg", bufs=4))
        qrp = ctx.enter_context(tc.tile_pool(name="qrp", bufs=3))
        zp = ctx.enter_context(tc.tile_pool(name="zp", bufs=3))
        mp = ctx.enter_context(tc.tile_pool(name="mp", bufs=3))
        trp = ctx.enter_context(tc.tile_pool(name="trp", bufs=DELAY + 2))
        fp = ctx.enter_context(tc.tile_pool(name="fp", bufs=DELAY + 2))
        apo = ctx.enter_context(tc.tile_pool(name="apo", bufs=3))

        # ---- constants ----
        wqkv_f = consts.tile([C, 3 * C], f32)
        nc.sync.dma_start(out=wqkv_f, in_=w_qkv[:, :])
        wqkv_bf = consts.tile([C, 3 * C], bf16)
        nc.vector.tensor_copy(wqkv_bf[:], wqkv_f[:])

        wp_f = consts.tile([C, C], f32)
        nc.sync.dma_start(out=wp_f, in_=w_proj[:, :])
        wp_bf = consts.tile([C, C], bf16)
        nc.vector.tensor_copy(wp_bf[:], wp_f[:])

        # row vectors (single partition) for the rank-1 bias-fold matmuls
        bqkv_row_f = consts.tile([1, 3 * C], f32)
        nc.sync.dma_start(out=bqkv_row_f, in_=b_qkv[:].unsqueeze(0))
        bqkv_row = consts.tile([1, 3 * C], bf16)
        nc.vector.tensor_copy(bqkv_row[:], bqkv_row_f[:])

        bp_row_f = consts.tile([1, C], f32)
        nc.sync.dma_start(out=bp_row_f, in_=b_proj[:].unsqueeze(0))
        bp_row = consts.tile([1, C], bf16)
        nc.vector.tensor_copy(bp_row[:], bp_row_f[:])

        ones_row = consts.tile([1, FL], bf16)
        nc.vector.memset(ones_row[:], 1.0)

        ident = consts.tile([128, 128], bf16)
        make_identity(nc, ident[:])

        rep_ctx = tc.For_i(0, repeat, 1) if repeat > 1 else nullcontext()
        with rep_ctx:
            _emit_flights(
                nc, mybir, qkvp, tpp, ypool, stg, qrp, zp, mp, trp, fp, apo,
                wqkv_bf, wp_bf, bqkv_row, bp_row, ones_row, ident, xT, outT,
            )

    nc.compile()
    return nc


DELAY = 2  # flights between a flight's head (softmax) and tail (proj+out)


def _emit_flights(
    nc, mybir, qkvp, tpp, ypool, stg, qrp, zp, mp, trp, fp, apo,
    wqkv_bf, wp_bf, bqkv_row, bp_row, ones_row, ident, xT, outT,
):
    f32 = mybir.dt.float32
    bf16 = mybir.dt.bfloat16
    Exp = mybir.ActivationFunctionType.Exp
    Copy = mybir.ActivationFunctionType.Copy

    def head(fi):
        t0 = fi * FL
        xt = stg.tile([C, FL], bf16, tag="xt")
        nc.sync.dma_start(out=xt, in_=xT[:, t0 : t0 + FL])

        k_sm = stg.tile([128, SUB, H, HD], bf16, tag="ksm")
        v_sm = stg.tile([128, SUB, H, HD], bf16, tag="vsm")
        qrep = qrp.tile([128, SH, HD, HD], bf16, tag="qrep")

        for s in range(SUB):
            # qkv matmul with bias folded in via a rank-1 PSUM init
            ps = qkvp.tile([128, 3, H, HD], f32, tag="ps")
            nc.tensor.matmul(
                out=ps[:],
                lhsT=ones_row[:, 0:128],
                rhs=bqkv_row[:],
                start=True,
                stop=False,
            )
            nc.tensor.matmul(
                out=ps[:],
                lhsT=xt[:, s * 128 : (s + 1) * 128],
                rhs=wqkv_bf[:],
                start=False,
                stop=True,
            )
            # ACT replicates q along j (PSUM f32 -> SBUF bf16, packed out)
            nc.scalar.activation(
                qrep[:, s * H : (s + 1) * H],
                ps[:, 0].unsqueeze(3).broadcast_to((128, H, HD, HD)),
                Copy,
            )
            # stage k, v (ACT) to SBUF bf16
            nc.scalar.copy(k_sm[:, s], ps[:, 1])
            nc.scalar.copy(v_sm[:, s], ps[:, 2])

        km = k_sm.rearrange("p s h j -> p (s h) j")
        k_ap = km.unsqueeze(2).broadcast_to((128, SH, HD, HD))

        # z[i,j] = q_i * k_j  (scale folded into exp); qrep packed -> 2x
        z = zp.tile([128, SH, HD, HD], bf16, tag="z")
        nc.vector.tensor_mul(z[:], qrep[:], k_ap)
        return z, v_sm

    def headB(state):
        z, v_sm = state
        vm = v_sm.rearrange("p s h j -> p (s h) j")
        v_ap = vm.unsqueeze(2).broadcast_to((128, SH, HD, HD))

        # M[:,0] = e = exp(scale*z) ; M[:,1] = ev = e * v
        M = mp.tile([128, 2, SH, HD, HD], bf16, tag="m")
        nc.scalar.activation(M[:, 0], z[:], Exp, scale=SCALE)
        nc.vector.tensor_mul(M[:, 1], M[:, 0], v_ap)

        # merged pairwise-add tree over j for both e (->D) and ev (->N)
        Mv = M.rearrange("p c s i j -> p (c s) i j")
        T1 = trp.tile([128, 2 * SH, HD, 8], bf16, tag="t1")
        nc.vector.tensor_add(T1[:], Mv[:, :, :, 0:8], Mv[:, :, :, 8:16])
        T2 = trp.tile([128, 2 * SH, HD, 4], bf16, tag="t2")
        nc.vector.tensor_add(T2[:], T1[:, :, :, 0:4], T1[:, :, :, 4:8])
        return T2

    def tail(fi, T2):
        t0 = fi * FL
        T3 = trp.tile([128, 2 * SH, HD, 2], bf16, tag="t3")
        nc.vector.tensor_add(T3[:], T2[:, :, :, 0:2], T2[:, :, :, 2:4])
        F = fp.tile([128, 2 * SH, HD], f32, tag="f")
        nc.gpsimd.tensor_add(F[:], T3[:, :, :, 0], T3[:, :, :, 1])
        # at = N / D  (recip on DVE, multiply on GPSIMD)
        rd = fp.tile([128, SH, HD], f32, tag="rd")
        nc.vector.reciprocal_approx_fast(out=rd[:], in_=F[:, 0:SH])
        at_a = apo.tile([128, SUB, C], bf16, tag="ata")
        nc.gpsimd.tensor_mul(
            at_a.rearrange("p s (h i) -> p (s h) i", h=H), F[:, SH : 2 * SH], rd[:]
        )

        # transpose to channel-major, proj matmul with bias folded, DMA out
        at_b = apo.tile([C, FL], bf16, tag="atb")
        for s in range(SUB):
            tp = tpp.tile([128, 128], bf16, tag="tp")
            nc.tensor.transpose(tp[:], at_a[:, s, :], ident[:])
            nc.vector.tensor_copy(at_b[:, s * 128 : (s + 1) * 128], tp[:])

        yp = ypool.tile([C, FL], f32, tag="yp")
        nc.tensor.matmul(
            out=yp[:], lhsT=bp_row[:], rhs=ones_row[:], start=True, stop=False
        )
        nc.tensor.matmul(
            out=yp[:], lhsT=wp_bf[:], rhs=at_b[:], start=False, stop=True
        )
        y = apo.tile([C, FL], f32, tag="y")
        nc.scalar.copy(y[:], yp[:])
        nc.sync.dma_start(out=outT[:, t0 : t0 + FL], in_=y[:])

    # 3-stage software pipeline: headA(f) | headB(f-1) | tail(f-2)
    pendB = []
    pendT = []
    for fi in range(NFL):
        pendB.append((fi, head(fi)))
        if len(pendB) > 1:
            g, st = pendB.pop(0)
            pendT.append((g, headB(st)))
        if len(pendT) >= DELAY:
            tail(*pendT.pop(0))
    while pendB:
        g, st = pendB.pop(0)
        pendT.append((g, headB(st)))
    while pendT:
        tail(*pendT.pop(0))


def kernel(x, w_qkv, b_qkv, w_proj, b_proj):
    from concourse import bass_utils

    global _BUILT
    if _BUILT is None:
        _BUILT = _build()
    nc = _BUILT

    import ml_dtypes

    xf = np.asarray(x, np.float32).reshape(NTOK, C)
    w_qkv = np.ascontiguousarray(np.asarray(w_qkv, np.float32))
    b_qkv = np.ascontiguousarray(np.asarray(b_qkv, np.float32))
    w_proj = np.ascontiguousarray(np.asarray(w_proj, np.float32))
    b_proj = np.ascontiguousarray(np.asarray(b_proj, np.float32))

    in_maps = []
    for i in range(NCORES):
        shard = xf[i * TPC : (i + 1) * TPC]
        in_maps.append(
            {
                "xT": np.ascontiguousarray(shard.T).astype(ml_dtypes.bfloat16),
                "w_qkv": w_qkv,
                "b_qkv": b_qkv,
                "w_proj": w_proj,
                "b_proj": b_proj,
            }
        )

    global _LAST_IN_MAPS
    _LAST_IN_MAPS = in_maps
    res = bass_utils.run_bass_kernel_spmd(nc, in_maps, core_ids=list(range(NCORES)))
    y = np.concatenate(
        [np.asarray(res.results[i]["outT"]).T for i in range(NCORES)], axis=0
    )
    return y.reshape(B, L, C)


# revision 9
# speedup vs baseline: 43.6964x; 1.0688x over previous
"""ChannelAttention TRN2 kernel.

Math (per token t, head h; hd=16):
  qkv = x @ w_qkv + b_qkv ; q,k,v = split(qkv)
  A[i,j] = softmax_j( scale * q[t,h,i] * k[t,h,j] )
  out[t,h,i] = sum_j A[i,j] v[t,h,j] ;  y = out @ w_proj + b_proj

65536 tokens sharded 8 ways, weights replicated, no collectives.
Tokens on SBUF partitions (128/tile), (h,i,j) along the free dim.

Engine split per 256-token flight (cost-model ns):
  PE   : rank-1 bias-fold matmuls + qkv matmul (per subtile), 2
         transposes, rank-1 proj-bias fold + proj matmul
  ACT  : q broadcast-replication along j (2x 1849), k/v staging copies
         PSUM->SBUF bf16, exp (3598), y copy
  DVE  : z=q*k 2x (2193), ev=e*v 2x (2193), merged e/ev pairwise tree
         L1 (2193) + L2 (1127) + L3 (593), recip (327), transpose copies
  Pool : tree L4 f32 (1110), at = N*(1/D) (603) — SBUF-only ops; the
         software GPSIMD runs well below its cost-model speed, so only
         the two smallest ops live here
All bf16 operands packed on the last dim so DVE TT ops hit 2x mode; the
replicated q operand of the outer product is materialized by ACT/Pool so
no stride-0 AP reaches DVE.  Biases ride into PSUM via rank-1 matmuls
(ones ⊗ bias) so no DVE bias adds exist.  Flights are software-pipelined
3 deep (headA | headB | tail) to kill cross-engine head-of-line stalls.
"""

import numpy as np

B, L, C = 4, 16384, 128
H, HD = 8, 16
NCORES = 8
NTOK = B * L
TPC = NTOK // NCORES  # 8192 tokens per core
SCALE = float(C) ** -0.5

FL = 256          # tokens per flight
SUB = FL // 128   # 128-token subtiles per flight
SH = SUB * H      # 16
NFL = TPC // FL   # 32

_BUILT = None
_LAST_IN_MAPS = None


def _build(repeat=1):
    import concourse.bass as bass
    from concourse import bacc
    from concourse import mybir
    from concourse.tile import TileContext
    from concourse.masks import make_identity
    from contextlib import ExitStack, nullcontext

    f32 = mybir.dt.float32
    bf16 = mybir.dt.bfloat16

    nc = bacc.Bacc("TRN2")
    xT = nc.dram_tensor("xT", [C, TPC], bf16, kind="ExternalInput")
    w_qkv = nc.dram_tensor("w_qkv", [C, 3 * C], f32, kind="ExternalInput")
    b_qkv = nc.dram_tensor("b_qkv", [3 * C], f32, kind="ExternalInput")
    w_proj = nc.dram_tensor("w_proj", [C, C], f32, kind="ExternalInput")
    b_proj = nc.dram_tensor("b_proj", [C], f32, kind="ExternalInput")
    outT = nc.dram_tensor("outT", [C, TPC], f32, kind="ExternalOutput")

    with TileContext(nc) as tc, ExitStack() as ctx:
        consts = ctx.enter_context(tc.tile_pool(name="consts", bufs=1))
        qkvp = ctx.enter_context(tc.tile_pool(name="qkvp", bufs=3, space="PSUM"))
        tpp = ctx.enter_context(tc.tile_pool(name="tpp", bufs=2, space="PSUM"))
        ypool = ctx.enter_context(tc.tile_pool(name="ypool", bufs=2, space="PSUM"))
        stg = ctx.enter_context(tc.tile_pool(name="stg", bufs=4))
        qrp = ctx.enter_context(tc.tile_pool(name="qrp", bufs=3))
        zp = ctx.enter_context(tc.tile_pool(name="zp", bufs=3))
        mp = ctx.enter_context(tc.tile_pool(name="mp", bufs=3))
        trp = ctx.enter_context(tc.tile_pool(name="trp", bufs=DELAY + 2))
        fp = ctx.enter_context(tc.tile_pool(name="fp", bufs=DELAY + 2))
        apo = ctx.enter_context(tc.tile_pool(name="apo", bufs=3))

        # ---- constants ----
        wqkv_f = consts.tile([C, 3 * C], f32)
        nc.sync.dma_start(out=wqkv_f, in_=w_qkv[:, :])
        wqkv_bf = consts.tile([C, 3 * C], bf16)
        nc.vector.tensor_copy(wqkv_bf[:], wqkv_f[:])

        wp_f = consts.tile([C, C], f32)
        nc.sync.dma_start(out=wp_f, in_=w_proj[:, :])
        wp_bf = consts.tile([C, C], bf16)
        nc.vector.tensor_copy(wp_bf[:], wp_f[:])

        # row vectors (single partition) for the rank-1 bias-fold matmuls
        bqkv_row_f = consts.tile([1, 3 * C], f32)
        nc.sync.dma_start(out=bqkv_row_f, in_=b_qkv[:].unsqueeze(0))
        bqkv_row = consts.tile([1, 3 * C], bf16)
        nc.vector.tensor_copy(bqkv_row[:], bqkv_row_f[:])

        bp_row_f = consts.tile([1, C], f32)
        nc.sync.dma_start(out=bp_row_f, in_=b_proj[:].unsqueeze(0))
        bp_row = consts.tile([1, C], bf16)
        nc.vector.tensor_copy(bp_row[:], bp_row_f[:])

        ones_row = consts.tile([1, FL], bf16)
        nc.vector.memset(ones_row[:], 1.0)

        ident = consts.tile([128, 128], bf16)
        make_identity(nc, ident[:])

        rep_ctx = tc.For_i(0, repeat, 1) if repeat > 1 else nullcontext()
        with rep_ctx:
            _emit_flights(
                nc, mybir, qkvp, tpp, ypool, stg, qrp, zp, mp, trp, fp, apo,
                wqkv_bf, wp_bf, bqkv_row, bp_row, ones_row, ident, xT, outT,
            )

    nc.compile()
    return nc


DELAY = 2  # flights between a flight's head (softmax) and tail (proj+out)


def _emit_flights(
    nc, mybir, qkvp, tpp, ypool, stg, qrp, zp, mp, trp, fp, apo,
    wqkv_bf, wp_bf, bqkv_row, bp_row, ones_row, ident, xT, outT,
):
    f32 = mybir.dt.float32
    bf16 = mybir.dt.bfloat16
    Exp = mybir.ActivationFunctionType.Exp
    Copy = mybir.ActivationFunctionType.Copy

    def head(fi):
        t0 = fi * FL
        xt = stg.tile([C, FL], bf16, tag="xt")
        nc.sync.dma_start(out=xt, in_=xT[:, t0 : t0 + FL])

        k_sm = stg.tile([128, SUB, H, HD], bf16, tag="ksm")
        v_sm = stg.tile([128, SUB, H, HD], bf16, tag="vsm")
        qrep = qrp.tile([128, SH, HD, HD], bf16, tag="qrep")

        for s in range(SUB):
            # qkv matmul with bias folded in via a rank-1 PSUM init
            ps = qkvp.tile([128, 3, H, HD], f32, tag="ps")
            nc.tensor.matmul(
                out=ps[:],
                lhsT=ones_row[:, 0:128],
                rhs=bqkv_row[:],
                start=True,
                stop=False,
            )
            nc.tensor.matmul(
                out=ps[:],
                lhsT=xt[:, s * 128 : (s + 1) * 128],
                rhs=wqkv_bf[:],
                start=False,
                stop=True,
            )
            # ACT replicates q along j (PSUM f32 -> SBUF bf16, packed out)
            nc.scalar.activation(
                qrep[:, s * H : (s + 1) * H],
                ps[:, 0].unsqueeze(3).broadcast_to((128, H, HD, HD)),
                Copy,
            )
            # stage k, v (ACT) to SBUF bf16
            nc.scalar.copy(k_sm[:, s], ps[:, 1])
            nc.scalar.copy(v_sm[:, s], ps[:, 2])

        km = k_sm.rearrange("p s h j -> p (s h) j")
        k_ap = km.unsqueeze(2).broadcast_to((128, SH, HD, HD))

        # z[i,j] = q_i * k_j  (scale folded into exp); qrep packed -> 2x
        z = zp.tile([128, SH, HD, HD], bf16, tag="z")
        nc.vector.tensor_mul(z[:], qrep[:], k_ap)
        return z, v_sm

    def headB(state):
        z, v_sm = state
        vm = v_sm.rearrange("p s h j -> p (s h) j")
        v_ap = vm.unsqueeze(2).broadcast_to((128, SH, HD, HD))

        # M[:,0] = e = exp(scale*z) ; M[:,1] = ev = e * v
        M = mp.tile([128, 2, SH, HD, HD], bf16, tag="m")
        nc.scalar.activation(M[:, 0], z[:], Exp, scale=SCALE)
        nc.vector.tensor_mul(M[:, 1], M[:, 0], v_ap)

        # merged pairwise-add tree over j for both e (->D) and ev (->N)
        Mv = M.rearrange("p c s i j -> p (c s) i j")
        T1 = trp.tile([128, 2 * SH, HD, 8], bf16, tag="t1")
        nc.vector.tensor_add(T1[:], Mv[:, :, :, 0:8], Mv[:, :, :, 8:16])
        T2 = trp.tile([128, 2 * SH, HD, 4], bf16, tag="t2")
        nc.vector.tensor_add(T2[:], T1[:, :, :, 0:4], T1[:, :, :, 4:8])
        return T2

    def tail(fi, T2):
        t0 = fi * FL
        T3 = trp.tile([128, 2 * SH, HD, 2], bf16, tag="t3")
        nc.gpsimd.tensor_add(T3[:], T2[:, :, :, 0:2], T2[:, :, :, 2:4])
        F = fp.tile([128, 2 * SH, HD], f32, tag="f")
        nc.gpsimd.tensor_add(F[:], T3[:, :, :, 0], T3[:, :, :, 1])
        # at = N / D  (recip on DVE, multiply on GPSIMD)
        rd = fp.tile([128, SH, HD], f32, tag="rd")
        nc.vector.reciprocal_approx_fast(out=rd[:], in_=F[:, 0:SH])
        at_a = apo.tile([128, SUB, C], bf16, tag="ata")
        nc.gpsimd.tensor_mul(
            at_a.rearrange("p s (h i) -> p (s h) i", h=H), F[:, SH : 2 * SH], rd[:]
        )

        # transpose to channel-major, proj matmul with bias folded, DMA out
        at_b = apo.tile([C, FL], bf16, tag="atb")
        for s in range(SUB):
            tp = tpp.tile([128, 128], bf16, tag="tp")
            nc.tensor.transpose(tp[:], at_a[:, s, :], ident[:])
            nc.vector.tensor_copy(at_b[:, s * 128 : (s + 1) * 128], tp[:])

        yp = ypool.tile([C, FL], f32, tag="yp")
        nc.tensor.matmul(
            out=yp[:], lhsT=bp_row[:], rhs=ones_row[:], start=True, stop=False
        )
        nc.tensor.matmul(
            out=yp[:], lhsT=wp_bf[:], rhs=at_b[:], start=False, stop=True
        )
        y = apo.tile([C, FL], f32, tag="y")
        nc.scalar.copy(y[:], yp[:])
        nc.sync.dma_start(out=outT[:, t0 : t0 + FL], in_=y[:])

    # 3-stage software pipeline: headA(f) | headB(f-1) | tail(f-2)
    pendB = []
    pendT = []
    for fi in range(NFL):
        pendB.append((fi, head(fi)))
        if len(pendB) > 1:
            g, st = pendB.pop(0)
            pendT.append((g, headB(st)))
        if len(pendT) >= DELAY:
            tail(*pendT.pop(0))
    while pendB:
        g, st = pendB.pop(0)
        pendT.append((g, headB(st)))
    while pendT:
        tail(*pendT.pop(0))


def kernel(x, w_qkv, b_qkv, w_proj, b_proj):
    from concourse import bass_utils

    global _BUILT
    if _BUILT is None:
        _BUILT = _build()
    nc = _BUILT

    import ml_dtypes

    xf = np.asarray(x, np.float32).reshape(NTOK, C)
    w_qkv = np.ascontiguousarray(np.asarray(w_qkv, np.float32))
    b_qkv = np.ascontiguousarray(np.asarray(b_qkv, np.float32))
    w_proj = np.ascontiguousarray(np.asarray(w_proj, np.float32))
    b_proj = np.ascontiguousarray(np.asarray(b_proj, np.float32))

    in_maps = []
    for i in range(NCORES):
        shard = xf[i * TPC : (i + 1) * TPC]
        in_maps.append(
            {
                "xT": np.ascontiguousarray(shard.T).astype(ml_dtypes.bfloat16),
                "w_qkv": w_qkv,
                "b_qkv": b_qkv,
                "w_proj": w_proj,
                "b_proj": b_proj,
            }
        )

    global _LAST_IN_MAPS
    _LAST_IN_MAPS = in_maps
    res = bass_utils.run_bass_kernel_spmd(nc, in_maps, core_ids=list(range(NCORES)))
    y = np.concatenate(
        [np.asarray(res.results[i]["outT"]).T for i in range(NCORES)], axis=0
    )
    return y.reshape(B, L, C)
